# revision 4
# baseline (speedup 1.0000x reference)
"""Correlation network kernel for Trainium2.

corr[b,i,j,k,l] = sum_c A[b,i,j,c] * B[b,k,l,c]

Per batch b this is  A_b (2304x64) @ B_b^T (64x2304) -> 2304x2304.
Sharding: data-parallel over batch B=8 across the 8 NeuronCores; each core
computes one full 2304x2304 correlation matrix, so the kernel is
output-write bound.

Device-side plan (per core):
  - Pure-bf16 compute with a bf16 DRAM output (upcast to fp32 on host).
    fro rel err ~3e-3 vs the fp32 reference (gate is 2e-2): bf16 input
    rounding ~2.4e-3 rms + bf16 output rounding ~1.1e-3 rms.  Halves the
    dominant HBM write (21.2 MB -> 10.6 MB/core) and cuts PE work 3x vs
    the previous hi/lo-split scheme.
  - Inputs arrive host-prepped in [C, HW] layout: lhsT packed [128, 1152]
    (rows 0:64 = even m-tiles, 64:128 = odd; K=C=64 so m-tiles pack in
    pairs into the 128-row PE array), rhs duplicated into both partition
    halves [128, 2304].  Loaded via the sync-engine HWDGE ring (the SP
    engine is otherwise idle until outputs start); the slow gpsimd SWDGE
    path (~670 ns/issue + late start) is avoided entirely.
  - Per (m-pair, 1024-col n-chunk): 4 bf16 matmuls (even rows 0:64 and
    odd rows 64:128, two 512-col PSUM banks each) into two 2-bank PSUM
    tiles, then one 1024-col PSUM fp32 -> SBUF bf16 copy per row
    (even rows on DVE, odd rows on ACT; Pool/gpsimd cannot read PSUM).
    2-bank copies amortize the ~250 ns per-instruction overhead.
  - Output DMAs ride the sync HWDGE ring as few large transfers (one
    589 KB contiguous row-block per m-tile in steady state; the first
    pair streams in 3 chunks to start the ring early, the last pair
    drains split across the sync + scalar rings).  DMA_DIRECT2D issue
    costs ~650 ns on the issuing engine, so fewer/larger beats many
    small.
"""

import numpy as np
import ml_dtypes

import concourse.bacc as bacc
import concourse.mybir as mybir
import concourse.tile as tile
from concourse.bass_interp import get_hw_module
from concourse.bass_utils import run_bass_kernel_spmd

B, H, W, C = 8, 48, 48, 64
HW = H * W  # 2304
P = 128
M_TILES = HW // P  # 18
M_PAIRS = M_TILES // 2  # 9
N_TILE = 512
FP32 = mybir.dt.float32
BF16 = mybir.dt.bfloat16
BF16_NP = ml_dtypes.bfloat16

N_SPLITS = []
_n0 = 0
while _n0 < HW:
    N_SPLITS.append((_n0, min(N_TILE, HW - _n0)))
    _n0 += N_TILE


# n-chunks per row block: 2-bank PSUM tiles so one copy drains 1024 cols
N_CHUNKS = [(0, 1024), (1024, 1024), (2048, 256)]


def _corr_body(tc, out, a_bf, b_bf):
    nc = tc.nc
    with (
        tc.tile_pool(name="ops", bufs=1) as op_pool,
        tc.tile_pool(name="ps", bufs=4, space="PSUM") as ps_pool,
        tc.tile_pool(name="outs", bufs=8) as out_pool,
    ):
        at = op_pool.tile([P, HW // 2], BF16)
        bt = op_pool.tile([P, HW], BF16)
        # Input loads on the sync HWDGE ring, ordered so the first
        # m-pair's operands land first and matmuls start early.
        nc.sync.dma_start(out=at[:, 0:P], in_=a_bf[:, 0:P])
        nc.sync.dma_start(out=bt[:, 0 : 2 * N_TILE], in_=b_bf[:, 0 : 2 * N_TILE])
        nc.sync.dma_start(out=at[:, P : HW // 2], in_=a_bf[:, P : HW // 2])
        nc.sync.dma_start(out=bt[:, 2 * N_TILE : HW], in_=b_bf[:, 2 * N_TILE : HW])

        for p in range(M_PAIRS):
            ot_e = out_pool.tile([P, HW], BF16, tag="ot")
            ot_o = out_pool.tile([P, HW], BF16, tag="ot")
            col = slice(p * P, (p + 1) * P)
            m_e, m_o = 2 * p, 2 * p + 1
            for ni, (n0, nsz) in enumerate(N_CHUNKS):
                ps_e = ps_pool.tile([P, 2 * N_TILE], FP32, tag="ps")
                ps_o = ps_pool.tile([P, 2 * N_TILE], FP32, tag="ps")
                for s0 in range(0, nsz, N_TILE):
                    ssz = min(N_TILE, nsz - s0)
                    nc.tensor.matmul(
                        ps_e[:, s0 : s0 + ssz],
                        at[0:64, col],
                        bt[0:64, n0 + s0 : n0 + s0 + ssz],
                        start=True,
                        stop=True,
                    )
                    nc.tensor.matmul(
                        ps_o[:, s0 : s0 + ssz],
                        at[64:128, col],
                        bt[64:128, n0 + s0 : n0 + s0 + ssz],
                        start=True,
                        stop=True,
                    )
                # even rows drain on DVE, odd rows on ACT: two independent
                # copy chains that run concurrently
                nc.vector.tensor_copy(ot_e[:, n0 : n0 + nsz], ps_e[:, :nsz])
                nc.scalar.copy(ot_o[:, n0 : n0 + nsz], ps_o[:, :nsz])

                if p == 0:
                    # First pair: stream each chunk to start the ring early.
                    c0, c1 = n0, n0 + nsz
                    nc.sync.dma_start(
                        out=out[m_e * P : (m_e + 1) * P, c0:c1],
                        in_=ot_e[:, c0:c1],
                    )
                    nc.sync.dma_start(
                        out=out[m_o * P : (m_o + 1) * P, c0:c1],
                        in_=ot_o[:, c0:c1],
                    )
                elif p < M_PAIRS - 1:
                    # Steady state: one contiguous 589 KB DMA per row block.
                    if ni == len(N_CHUNKS) - 1:
                        nc.sync.dma_start(
                            out=out[m_e * P : (m_e + 1) * P, :], in_=ot_e[:, :]
                        )
                        nc.sync.dma_start(
                            out=out[m_o * P : (m_o + 1) * P, :], in_=ot_o[:, :]
                        )
                else:
                    # Last pair: drain split across sync + scalar rings.
                    if ni >= 1:
                        c0 = 0 if ni == 1 else 2048
                        c1 = n0 + nsz
                        nc.sync.dma_start(
                            out=out[m_e * P : (m_e + 1) * P, c0:c1],
                            in_=ot_e[:, c0:c1],
                        )
                        nc.scalar.dma_start(
                            out=out[m_o * P : (m_o + 1) * P, c0:c1],
                            in_=ot_o[:, c0:c1],
                        )


_NC_CACHE = None


def _build():
    global _NC_CACHE
    if _NC_CACHE is None:
        nc = bacc.Bacc(
            "TRN2",
            target_bir_lowering=False,
            debug=False,
            enable_asserts=False,
        )
        a_bf = nc.dram_tensor("a_bf", [P, HW // 2], BF16, kind="ExternalInput").ap()
        b_bf = nc.dram_tensor("b_bf", [P, HW], BF16, kind="ExternalInput").ap()
        out = nc.dram_tensor("out", [HW, HW], BF16, kind="ExternalOutput").ap()
        with tile.TileContext(nc) as tc:
            _corr_body(tc, out, a_bf, b_bf)
        nc.compile()
        nc.m = get_hw_module(nc.m)
        _NC_CACHE = nc
    return _NC_CACHE


def _pack_lhs(xT):
    """[C, HW] -> [128, HW/2]: rows 0:64 even m-tiles, rows 64:128 odd."""
    t = xT.reshape(C, M_PAIRS, 2, P)  # [c, pair, eo, j]
    return np.ascontiguousarray(t.transpose(2, 0, 1, 3).reshape(2 * C, M_PAIRS * P))


def _pack_rhs(xT):
    """[C, HW] -> [128, HW]: duplicate into both partition halves."""
    return np.ascontiguousarray(np.concatenate([xT, xT], axis=0))


def _prep_inputs(feature_A, feature_B):
    in_maps = []
    for i in range(B):
        A2 = feature_A[i].reshape(HW, C).astype(BF16_NP)
        B2 = feature_B[i].reshape(HW, C).astype(BF16_NP)
        in_maps.append(
            {
                "a_bf": _pack_lhs(np.ascontiguousarray(A2.T)),
                "b_bf": _pack_rhs(np.ascontiguousarray(B2.T)),
            }
        )
    return in_maps


def _run(feature_A, feature_B, trace=False, **kwargs):
    feature_A = np.asarray(feature_A, dtype=np.float32)
    feature_B = np.asarray(feature_B, dtype=np.float32)
    assert feature_A.shape == (B, H, W, C), feature_A.shape
    assert feature_B.shape == (B, H, W, C), feature_B.shape

    nc = _build()
    in_maps = _prep_inputs(feature_A, feature_B)
    res = run_bass_kernel_spmd(nc, in_maps, list(range(B)), trace=trace, **kwargs)
    out = np.stack(
        [np.asarray(res.results[i]["out"]).astype(np.float32) for i in range(B)],
        axis=0,
    )
    return out.reshape(B, H, W, H, W), res


def kernel(feature_A, feature_B):
    out, _ = _run(feature_A, feature_B)
    return out
